# revision 39
# baseline (speedup 1.0000x reference)
"""Causal multi-head attention (B=2, S=2048, H=1024, 16 heads, hd=64) on 8
Trainium2 NeuronCores.

Sharding: batch x head-group. Core c handles batch c//4 and the 4 heads
4*(c%4)..4*(c%4)+3 (a 256-wide column slice of Q/K/V). Each core computes its
heads' contribution to the output projection (row-parallel Wo); the host sums
the 4 partials per batch and adds bo.

Per-core kernel (all matmuls in bf16, fp32 PSUM accumulation):
  phase 1: a short K=1 warm-up matmul burst runs while the first DMAs land
           (HAM clock). q/k projections run kc-major (4 PSUM banks hold the
           4 token blocks) so each whole-row xt DMA is consumed as it
           arrives; all DMA layouts are host-prepared so per-partition runs
           are 4KB. xt stays resident in SBUF for the whole kernel.
  phase 2: per (head, 512-query block): scoresT[k,q] tiles on PE (causal:
           only k-blocks <= block end). Diagonal k-tiles are computed only
           on their valid query range (free-dim sliced), so masking reduces
           to one 128x128 triangle add on DVE per diagonal tile; exp on ACT
           (scale=1/8 folded in; scores are bounded ~|3.8| so no
           max-subtraction is needed), then ctxT[65,q] = v_aug.T @ expT
           accumulated on PE - v_aug carries a ones column so row 64 is the
           softmax denominator. Reciprocal row is broadcast across 64
           partitions with a K=1 matmul and applied on DVE, writing
           normalized ctxT straight into the outproj stationary layout.
  phase 3: out_partial[tok,1024] = ctxT.T @ WoT-slice, streamed to DRAM.
"""
import ml_dtypes
import numpy as np

import concourse.bass as bass
import concourse.mybir as mybir
import concourse.tile as tile
from concourse.bass import ts
from concourse.bass_utils import run_bass_kernel_spmd

B, S, H, NH, HD = 2, 2048, 1024, 16, 64
NCORES = 8
HPC = 4            # heads per core
HSW = HPC * HD     # 256: head-slice width
F32 = mybir.dt.float32
BF16 = mybir.dt.bfloat16
NEG = -1.0e9


def _split_multi_waits(nc) -> int:
    """This walrus accepts at most ONE sync wait per instruction. Split any
    multi-wait instruction into single-wait NOPs (same engine, just before
    it) + the instruction carrying the last wait. Equivalent semantics:
    waits run in program order on the engine's queue."""
    n = 0
    for f in nc.m.functions:
        for blk in f.blocks:
            new_insts = []
            for inst in blk.instructions:
                si = inst.sync_info
                if si is not None and si.on_wait and len(si.on_wait) > 1:
                    waits = list(si.on_wait)
                    for i, w in enumerate(waits[:-1]):
                        new_insts.append(mybir.InstNoOp(
                            name=f"{inst.name}-ws{i}",
                            engine=inst.engine,
                            bass_nofuse=True,
                            sync_info=mybir.SyncInfo(on_wait=[w], on_update=[]),
                        ))
                        n += 1
                    si.on_wait = [waits[-1]]
                new_insts.append(inst)
            blk.instructions[:] = new_insts
    return n


def _build():
    nc = bass.Bass()
    xt_d = nc.dram_tensor("xt", [H, S], BF16, kind="ExternalInput")
    wq_d = nc.dram_tensor("wq", [128, 8, HSW], BF16, kind="ExternalInput")
    wk_d = nc.dram_tensor("wk", [128, 8, HSW], BF16, kind="ExternalInput")
    wv_d = nc.dram_tensor("wv", [128, 8, HSW], BF16, kind="ExternalInput")
    wo_d = nc.dram_tensor("wo", [128, 2, H], BF16, kind="ExternalInput")
    vb_d = nc.dram_tensor("vb", [128, HSW], F32, kind="ExternalInput")
    bqkvt_d = nc.dram_tensor("bqkvt", [128, 2, 2], F32, kind="ExternalInput")
    mb_d = nc.dram_tensor("mb", [128, 128], F32, kind="ExternalInput")
    out_d = nc.dram_tensor("out", [S, H], BF16, kind="ExternalOutput")

    EXP = mybir.ActivationFunctionType.Exp
    NQB = S // 512      # 4 query blocks per head
    NTC = S // 128      # 16 token chunks

    with tile.TileContext(nc) as tc:
        with tc.tile_pool(name="const", bufs=1) as constp, \
             tc.tile_pool(name="persist", bufs=1) as pers:
            wq = constp.tile([128, 8, HSW], BF16)
            wk = constp.tile([128, 8, HSW], BF16)
            wv = constp.tile([128, 8, HSW], BF16)
            wo = constp.tile([128, 2, H], BF16)
            vb = constp.tile([128, HSW], F32)      # v bias, row-replicated
            bqkvt = constp.tile([128, 2, 2], F32)  # [p, mc, q|k] per-row bias
            mbt = constp.tile([128, 128], F32)
            onesf = constp.tile([128, 512], F32)
            nc.vector.memset(onesf, 1.0)
            ones = constp.tile([1, 512], BF16)
            nc.vector.tensor_copy(out=ones, in_=onesf[0:1, :])

            xt = pers.tile([128, 8, S], BF16)     # resident for whole kernel
            qT = pers.tile([128, 2, S], BF16)     # [2 heads x 64 hd, mchunk, tok]
            kT = pers.tile([128, 2, S], BF16)
            vaug = pers.tile([128, 4, NTC, HD + 1], BF16)  # [ktok, head, kchunk, hd|1]
            ctxT = pers.tile([128, 2, S], BF16)   # outproj stationary layout
            nc.vector.tensor_copy(
                out=vaug[:, :, :, HD:HD + 1],
                in_=onesf[:, 0:64].rearrange("p (a b o) -> p a b o", a=4, b=16))

            # ---- phase 1: projections ----
            with tc.tile_pool(name="ps1", bufs=4, space="PSUM") as ps1, \
                 tc.tile_pool(name="ps1v", bufs=3, space="PSUM") as ps1v:
                # xt whole rows per kc: contiguous 4KB per-partition runs.
                # kc=0 + the kc=0 slice of wq first so the q-pass can start
                # ASAP. No PE warm-up: the q-pass is DMA-bound either way, so
                # let the HAM warm window start at the dense k-pass instead
                # (the ~75us full-clock power budget then covers the densest
                # stream).
                nc.sync.dma_start(out=wq[:, 0, :], in_=wq_d[:, 0, :])
                nc.sync.dma_start(out=xt[:, 0, :], in_=xt_d[ts(0, 128), :])
                nc.sync.dma_start(out=wq[:, 1:8, :], in_=wq_d[:, 1:8, :])
                for kc in range(1, 8):
                    nc.sync.dma_start(out=xt[:, kc, :], in_=xt_d[ts(kc, 128), :])
                nc.sync.dma_start(out=wk, in_=wk_d[:, :, :])
                nc.sync.dma_start(out=bqkvt, in_=bqkvt_d[:, :, :])
                nc.sync.dma_start(out=wv, in_=wv_d[:, :, :])
                nc.sync.dma_start(out=vb, in_=vb_d[:, :])
                nc.sync.dma_start(out=mbt, in_=mb_d[:, :])
                nc.sync.dma_start(out=wo, in_=wo_d[:, :, :])

                # q/k projections kc-major: each whole-row xt DMA is consumed
                # as it arrives; 4 PSUM banks hold the 4 token blocks.
                for w, brow, dst in ((wq, 0, qT), (wk, 1, kT)):
                    pb = [ps1.tile([128, 512], F32, tag="ps1", name="pb")
                          for _ in range(NQB)]
                    for kc in range(8):
                        for nb in range(NQB):
                            nc.tensor.matmul(pb[nb], w[:, kc, 0:128],
                                             xt[:, kc, ts(nb, 512)],
                                             start=(kc == 0), stop=(kc == 7))
                    for nb in range(NQB):
                        # bias folded into the PSUM->SBUF cast (per-partition
                        # scalar add on DVE)
                        nc.vector.tensor_scalar_add(
                            out=dst[:, 0, ts(nb, 512)], in0=pb[nb],
                            scalar1=bqkvt[:, 0, brow:brow + 1])

                for t in range(NTC):
                    ps = ps1v.tile([128, HSW], F32, tag="psv", name="ps")
                    for kc in range(8):
                        nc.tensor.matmul(ps, xt[:, kc, ts(t, 128)], wv[:, kc, :],
                                         start=(kc == 0), stop=(kc == 7))
                    for h in range(HPC):
                        # v bias folded into the PSUM->SBUF cast on DVE
                        nc.vector.tensor_add(vaug[:, h, t, 0:HD],
                                             ps[:, ts(h, HD)], vb[:, ts(h, HD)])

            # ---- phase 2+3: attention with software-pipelined epilogues ----
            # Heads processed singly (qb outer). The normalization epilogue of
            # slot (qb, h) -- reciprocal via ACT exp(-ln d), PE broadcast, DVE
            # scale -- is DEFERRED until after the NEXT slot's score/ctx groups
            # are emitted: the static per-engine schedule then has the next
            # slot's matmuls between ctx-stop and the broadcast matmul, so the
            # PE never idles waiting on the reciprocal chain. Outproj for qb is
            # deferred two slots for the same reason.
            # Diagonal k-tiles are free-dim sliced to their valid query range
            # [128j, 512): only a 128x128 triangle add remains for masking.
            # PSUM: pss 2x2 + ctx 2 + misc 1 + ps1b 1 = 8 banks.
            with tc.tile_pool(name="pss", bufs=2, space="PSUM") as pss, \
                 tc.tile_pool(name="psc", bufs=2, space="PSUM") as psc, \
                 tc.tile_pool(name="psm", bufs=1, space="PSUM") as psm, \
                 tc.tile_pool(name="ps1b", bufs=1, space="PSUM") as ps1b, \
                 tc.tile_pool(name="attnp", bufs=3) as attnp, \
                 tc.tile_pool(name="outp", bufs=3) as outp:

                # Interleaved heads-2/3 q/k projection chunks: dense 9-matmul
                # accumulation runs that keep the PE HAM activity window busy
                # (warm clock) between attention slots. xt is resident, so no
                # re-DMA is needed.
                def qk_mc1_chunk(w, brow, dst, nb):
                    def run():
                        ps = ps1b.tile([128, 512], F32, tag="ps1b", name="ps1b")
                        for kc in range(8):
                            nc.tensor.matmul(ps, w[:, kc, 128:256],
                                             xt[:, kc, ts(nb, 512)],
                                             start=(kc == 0), stop=(kc == 7))
                        nc.vector.tensor_scalar_add(
                            out=dst[:, 1, ts(nb, 512)], in0=ps,
                            scalar1=bqkvt[:, 1, brow:brow + 1])
                    return run
                qk_units = [qk_mc1_chunk(w, brow, dst, nb)
                            for (w, brow, dst) in ((wq, 0, qT), (wk, 1, kT))
                            for nb in range(NQB)]



                def norm_epilogue(qb, h, cps):
                    def run():
                        mc, ro = h // 2, (h % 2) * HD
                        lnr = attnp.tile([1, 512], F32, tag="lnr", name="lnr")
                        nc.scalar.activation(out=lnr, in_=cps[HD:HD + 1, :],
                                             func=mybir.ActivationFunctionType.Ln)
                        rec = attnp.tile([1, 512], BF16, tag="rec", name="rec")
                        nc.scalar.activation(out=rec, in_=lnr, func=EXP,
                                             scale=-1.0)
                        bps = psm.tile([128, 512], F32, tag="m", name="bps")
                        nc.tensor.matmul(bps[0:HD, :], ones[0:1, 0:HD],
                                         rec[0:1, :], start=True, stop=True)
                        bsb = attnp.tile([HD, 512], BF16, tag="bsb", name="bsb")
                        nc.vector.tensor_copy(out=bsb, in_=bps[0:HD, :])
                        nc.vector.tensor_mul(
                            out=ctxT[ro:ro + HD, mc, ts(qb, 512)],
                            in0=cps[0:HD, :], in1=bsb)
                    return run

                def outproj(qb, half=None):
                    t0, t1 = 4 * qb, 4 * qb + 4
                    if half is not None:
                        t0, t1 = t0 + 2 * half, t0 + 2 * half + 2
                    def run():
                        for t in range(t0, t1):
                            osb = outp.tile([128, H], BF16, tag="osb", name="osb")
                            for n2 in range(2):
                                # Alternate across two PSUM banks (ps1b is idle
                                # once the qk bursts are done) so each outproj
                                # blob is a dense 8-matmul run - it both flows
                                # faster and re-warms the HAM clock.
                                ops = (psm if n2 == 0 else ps1b).tile(
                                    [128, 512], F32,
                                    tag="m" if n2 == 0 else "ps1b", name="ops")
                                nc.tensor.matmul(ops, ctxT[:, 0, ts(t, 128)],
                                                 wo[:, 0, ts(n2, 512)],
                                                 start=True, stop=False)
                                nc.tensor.matmul(ops, ctxT[:, 1, ts(t, 128)],
                                                 wo[:, 1, ts(n2, 512)],
                                                 start=False, stop=True)
                                nc.vector.tensor_copy(out=osb[:, ts(n2, 512)],
                                                      in_=ops)
                            nc.sync.dma_start(out=out_d[ts(t, 128), :], in_=osb)
                    return run

                # Slots run in DESCENDING qb order: the PE-dense big slots
                # execute early (inside the HAM warm-clock power budget), the
                # relatively ACT-heavy small slots land in the throttled
                # window, and the big outproj blocks flush mid-kernel instead
                # of at the tail.
                # Per-slot PE filler schedule: the 8 heads-2/3 qk chunks go on
                # the thin sec-1 slots (every slot is ACT-deficient at warm
                # clock). Unit q_nb/k_nb is needed before slot (nb,2).
                uq = {nb: qk_units[nb] for nb in range(NQB)}
                uk = {nb: qk_units[NQB + nb] for nb in range(NQB)}
                fillers = {
                    (2, 0): uq[3], (2, 1): uk[3], (1, 0): uq[2],
                    (1, 1): uk[2], (0, 0): uq[1], (0, 1): uk[1],
                }
                extra = {(0, 0): uq[0], (0, 1): uk[0]}
                deferred = []
                for qb, h in ([(q, hh) for q in reversed(range(NQB))
                               for hh in (0, 1)]
                              + [(q, hh) for q in reversed(range(NQB))
                                 for hh in (2, 3)]):
                    last_kb = 4 * qb + 3
                    if True:
                        mc, ro = h // 2, (h % 2) * HD
                        cps = psc.tile([128, 512], F32, tag="ctx", name="cps")
                        # Emit group PAIRS: 4 scores mms, 2 exps, 4 ctx mms.
                        # The PE stream becomes continuous ~8-matmul dense runs
                        # (scores of pair N overlap exp of pair N-1), which
                        # keeps the HAM activity window busy (full clock).
                        for gp in range(qb + 1):
                            diag = (gp == qb)
                            sets = []
                            for g in (2 * gp, 2 * gp + 1):
                                sps = pss.tile([128, 2, 512], F32, tag="s",
                                               name="sps")
                                et = attnp.tile([128, 2, 512], BF16, tag="et",
                                                name="et")
                                sets.append((g, sps, et))
                                for u in range(2):
                                    kb = 2 * g + u
                                    j = kb - 4 * qb
                                    lo = 128 * j if j > 0 else 0
                                    nc.tensor.matmul(
                                        sps[:, u, lo:512],
                                        kT[ro:ro + HD, mc, ts(kb, 128)],
                                        qT[ro:ro + HD, mc,
                                           qb * 512 + lo:(qb + 1) * 512],
                                        start=True, stop=True)
                                    if j >= 0:
                                        nc.vector.tensor_add(
                                            sps[:, u, 128 * j:128 * j + 128],
                                            sps[:, u, 128 * j:128 * j + 128],
                                            mbt)
                            for g, sps, et in sets:
                                if diag and g % 2 == 1:
                                    # j=2,3 tiles: one ACT over the union
                                    # range [256:512). The j=3 garbage cols
                                    # [256:384) hold stale scores (bounded),
                                    # exp of them is finite and never read.
                                    nc.scalar.activation(
                                        out=et[:, :, 256:512],
                                        in_=sps[:, :, 256:512],
                                        func=EXP, scale=0.125)
                                else:
                                    # off-diag, or diag j=0,1 (union is the
                                    # full range)
                                    nc.scalar.activation(out=et, in_=sps,
                                                         func=EXP, scale=0.125)
                            for g, sps, et in sets:
                                for u in range(2):
                                    kb = 2 * g + u
                                    j = kb - 4 * qb
                                    lo = 128 * j if j > 0 else 0
                                    nc.tensor.matmul(
                                        cps[0:HD + 1, lo:512],
                                        vaug[:, h, kb, :],
                                        et[:, u, lo:512],
                                        start=(kb == 0),
                                        stop=(kb == last_kb),
                                        skip_group_check=True)
                        # flush one pending epilogue, then defer this slot's
                        while len(deferred) >= 2:
                            deferred.pop(0)()
                        # outproj halves (2 token chunks each) for qb+1 (whose
                        # norms have all flushed by now in descending order)
                        # queued BEFORE this slot's norm: the flush spreads
                        # their PE-dense work across the following slots.
                        if h == 3 and qb < NQB - 1:
                            deferred.append(outproj(qb + 1, 0))
                            deferred.append(outproj(qb + 1, 1))
                        deferred.append(norm_epilogue(qb, h, cps))
                        f = fillers.pop((qb, h), None)
                        if f is not None:
                            f()
                        f = extra.pop((qb, h), None)
                        if f is not None:
                            f()
                for fn in deferred:
                    fn()
                outproj(0, 0)()
                outproj(0, 1)()

    _split_multi_waits(nc)
    return nc


_NC_CACHE = []


def _get_nc():
    if not _NC_CACHE:
        _NC_CACHE.append(_build())
    return _NC_CACHE[0]


def _triangle_mask() -> np.ndarray:
    """mbt[p, f] = 0 where p <= f (key p attends to query f), else NEG.
    Applied to the 128x128 leading-diagonal corner of each diagonal k-tile."""
    p = np.arange(128)[:, None]
    f = np.arange(128)[None, :]
    return np.where(p <= f, 0.0, NEG).astype(np.float32)


def _in_maps(inputs: dict) -> list[dict]:
    bf16 = ml_dtypes.bfloat16
    x = np.asarray(inputs["hidden_states"], dtype=np.float32).astype(bf16)
    Wq = np.asarray(inputs["Wq"], dtype=np.float32).astype(bf16)
    Wk = np.asarray(inputs["Wk"], dtype=np.float32).astype(bf16)
    Wv = np.asarray(inputs["Wv"], dtype=np.float32).astype(bf16)
    Wo = np.asarray(inputs["Wo"], dtype=np.float32).astype(bf16)
    bq = np.asarray(inputs["bq"], dtype=np.float32).astype(bf16)
    bk = np.asarray(inputs["bk"], dtype=np.float32).astype(bf16)
    bv = np.asarray(inputs["bv"], dtype=np.float32).astype(bf16)

    xts = [np.ascontiguousarray(x[b].T) for b in range(B)]
    mbt = _triangle_mask()

    def wlayout(wt, c):
        # [c*128, n] -> [128, c, n] so per-partition DMA runs are contiguous
        return np.ascontiguousarray(
            wt.reshape(c, 128, wt.shape[1]).transpose(1, 0, 2))

    bqf = np.asarray(inputs["bq"], dtype=np.float32)
    bkf = np.asarray(inputs["bk"], dtype=np.float32)
    bvf = np.asarray(inputs["bv"], dtype=np.float32)
    maps = []
    for c in range(NCORES):
        b, hg = c // 4, c % 4
        hs = slice(hg * HSW, (hg + 1) * HSW)
        # [p, mc, q|k] fp32 per-row bias for the DVE tensor_scalar add
        bqkvt = np.ascontiguousarray(
            np.stack([bqf[hs].reshape(2, 128), bkf[hs].reshape(2, 128)],
                     axis=-1).transpose(1, 0, 2))
        maps.append({
            "xt": xts[b],
            "wq": wlayout(np.ascontiguousarray(Wq[hs, :].T), 8),
            "wk": wlayout(np.ascontiguousarray(Wk[hs, :].T), 8),
            "wv": wlayout(np.ascontiguousarray(Wv[hs, :].T), 8),
            "wo": wlayout(np.ascontiguousarray(Wo[:, hs].T), 2),
            "vb": np.ascontiguousarray(
                np.broadcast_to(bvf[hs][None, :], (128, HSW))),
            "bqkvt": bqkvt,
            "mb": mbt,
        })
    return maps


def run(inputs: dict, **spmd_kwargs):
    """Returns (full_output, BassKernelResults)."""
    nc = _get_nc()
    res = run_bass_kernel_spmd(nc, _in_maps(inputs), list(range(NCORES)),
                               **spmd_kwargs)
    bo = np.asarray(inputs["bo"], dtype=np.float32)
    out = np.empty((B, S, H), dtype=np.float32)
    for b in range(B):
        acc = res.results[4 * b]["out"].astype(np.float32)
        for hg in range(1, 4):
            acc = acc + res.results[4 * b + hg]["out"].astype(np.float32)
        out[b] = acc + bo
    return out, res


def kernel(**inputs) -> np.ndarray:
    out, _ = run(inputs)
    return out


# revision 45
# speedup vs baseline: 1.0044x; 1.0044x over previous
"""Causal multi-head attention (B=2, S=2048, H=1024, 16 heads, hd=64) on 8
Trainium2 NeuronCores.

Sharding: batch x head-group. Core c handles batch c//4 and the 4 heads
4*(c%4)..4*(c%4)+3 (a 256-wide column slice of Q/K/V). Each core computes its
heads' contribution to the output projection (row-parallel Wo); the host sums
the 4 partials per batch and adds bo.

Per-core kernel (all matmuls in bf16, fp32 PSUM accumulation):
  phase 1: a short K=1 warm-up matmul burst runs while the first DMAs land
           (HAM clock). q/k projections run kc-major (4 PSUM banks hold the
           4 token blocks) so each whole-row xt DMA is consumed as it
           arrives; all DMA layouts are host-prepared so per-partition runs
           are 4KB. xt stays resident in SBUF for the whole kernel.
  phase 2: per (head, 512-query block): scoresT[k,q] tiles on PE (causal:
           only k-blocks <= block end). Diagonal k-tiles are computed only
           on their valid query range (free-dim sliced), so masking reduces
           to one 128x128 triangle add on DVE per diagonal tile; exp on ACT
           (scale=1/8 folded in; scores are bounded ~|3.8| so no
           max-subtraction is needed), then ctxT[65,q] = v_aug.T @ expT
           accumulated on PE - v_aug carries a ones column so row 64 is the
           softmax denominator. Reciprocal row is broadcast across 64
           partitions with a K=1 matmul and applied on DVE, writing
           normalized ctxT straight into the outproj stationary layout.
  phase 3: out_partial[tok,1024] = ctxT.T @ WoT-slice, streamed to DRAM.
"""
import ml_dtypes
import numpy as np

import concourse.bass as bass
import concourse.mybir as mybir
import concourse.tile as tile
from concourse.bass import ts
from concourse.bass_utils import run_bass_kernel_spmd

B, S, H, NH, HD = 2, 2048, 1024, 16, 64
NCORES = 8
HPC = 4            # heads per core
HSW = HPC * HD     # 256: head-slice width
F32 = mybir.dt.float32
BF16 = mybir.dt.bfloat16
NEG = -1.0e9


def _split_multi_waits(nc) -> int:
    """This walrus accepts at most ONE sync wait per instruction. Split any
    multi-wait instruction into single-wait NOPs (same engine, just before
    it) + the instruction carrying the last wait. Equivalent semantics:
    waits run in program order on the engine's queue."""
    n = 0
    for f in nc.m.functions:
        for blk in f.blocks:
            new_insts = []
            for inst in blk.instructions:
                si = inst.sync_info
                if si is not None and si.on_wait and len(si.on_wait) > 1:
                    waits = list(si.on_wait)
                    for i, w in enumerate(waits[:-1]):
                        new_insts.append(mybir.InstNoOp(
                            name=f"{inst.name}-ws{i}",
                            engine=inst.engine,
                            bass_nofuse=True,
                            sync_info=mybir.SyncInfo(on_wait=[w], on_update=[]),
                        ))
                        n += 1
                    si.on_wait = [waits[-1]]
                new_insts.append(inst)
            blk.instructions[:] = new_insts
    return n


def _build():
    nc = bass.Bass()
    xt_d = nc.dram_tensor("xt", [H, S], BF16, kind="ExternalInput")
    wq_d = nc.dram_tensor("wq", [128, 8, HSW], BF16, kind="ExternalInput")
    wk_d = nc.dram_tensor("wk", [128, 8, HSW], BF16, kind="ExternalInput")
    wv_d = nc.dram_tensor("wv", [128, 8, HSW], BF16, kind="ExternalInput")
    wo_d = nc.dram_tensor("wo", [128, 2, H], BF16, kind="ExternalInput")
    vb_d = nc.dram_tensor("vb", [128, HSW], F32, kind="ExternalInput")
    bqkvt_d = nc.dram_tensor("bqkvt", [128, 2, 2], F32, kind="ExternalInput")
    mb_d = nc.dram_tensor("mb", [128, 128], F32, kind="ExternalInput")
    out_d = nc.dram_tensor("out", [S, H], BF16, kind="ExternalOutput")

    EXP = mybir.ActivationFunctionType.Exp
    NQB = S // 512      # 4 query blocks per head
    NTC = S // 128      # 16 token chunks

    with tile.TileContext(nc) as tc:
        with tc.tile_pool(name="const", bufs=1) as constp, \
             tc.tile_pool(name="persist", bufs=1) as pers:
            wq = constp.tile([128, 8, HSW], BF16)
            wk = constp.tile([128, 8, HSW], BF16)
            wv = constp.tile([128, 8, HSW], BF16)
            wo = constp.tile([128, 2, H], BF16)
            vb = constp.tile([128, HSW], F32)      # v bias, row-replicated
            bqkvt = constp.tile([128, 2, 2], F32)  # [p, mc, q|k] per-row bias
            mbt = constp.tile([128, 128], F32)
            onesf = constp.tile([128, 512], F32)
            nc.vector.memset(onesf, 1.0)
            ones = constp.tile([1, 512], BF16)
            nc.vector.tensor_copy(out=ones, in_=onesf[0:1, :])

            xt = pers.tile([128, 8, S], BF16)     # resident for whole kernel
            qT = pers.tile([128, 2, S], BF16)     # [2 heads x 64 hd, mchunk, tok]
            kT = pers.tile([128, 2, S], BF16)
            vaug = pers.tile([128, 4, NTC, HD + 1], BF16)  # [ktok, head, kchunk, hd|1]
            ctxT = pers.tile([128, 2, S], BF16)   # outproj stationary layout
            nc.vector.tensor_copy(
                out=vaug[:, :, :, HD:HD + 1],
                in_=onesf[:, 0:64].rearrange("p (a b o) -> p a b o", a=4, b=16))

            # ---- phase 1: projections ----
            with tc.tile_pool(name="ps1", bufs=4, space="PSUM") as ps1, \
                 tc.tile_pool(name="ps1v", bufs=3, space="PSUM") as ps1v:
                # xt whole rows per kc: contiguous 4KB per-partition runs.
                # kc=0 + the kc=0 slice of wq first so the q-pass can start
                # ASAP. No PE warm-up: the q-pass is DMA-bound either way, so
                # let the HAM warm window start at the dense k-pass instead
                # (the ~75us full-clock power budget then covers the densest
                # stream).
                nc.sync.dma_start(out=wq[:, 0, :], in_=wq_d[:, 0, :])
                nc.sync.dma_start(out=xt[:, 0, :], in_=xt_d[ts(0, 128), :])
                nc.sync.dma_start(out=wq[:, 1:8, :], in_=wq_d[:, 1:8, :])
                for kc in range(1, 8):
                    nc.sync.dma_start(out=xt[:, kc, :], in_=xt_d[ts(kc, 128), :])
                nc.sync.dma_start(out=wk, in_=wk_d[:, :, :])
                nc.sync.dma_start(out=bqkvt, in_=bqkvt_d[:, :, :])
                nc.sync.dma_start(out=wv, in_=wv_d[:, :, :])
                nc.sync.dma_start(out=vb, in_=vb_d[:, :])
                nc.sync.dma_start(out=mbt, in_=mb_d[:, :])
                nc.sync.dma_start(out=wo, in_=wo_d[:, :, :])

                # q/k projections kc-major: each whole-row xt DMA is consumed
                # as it arrives; 4 PSUM banks hold the 4 token blocks.
                for w, brow, dst in ((wq, 0, qT), (wk, 1, kT)):
                    pb = [ps1.tile([128, 512], F32, tag="ps1", name="pb")
                          for _ in range(NQB)]
                    for kc in range(8):
                        for nb in range(NQB):
                            nc.tensor.matmul(pb[nb], w[:, kc, 0:128],
                                             xt[:, kc, ts(nb, 512)],
                                             start=(kc == 0), stop=(kc == 7))
                    for nb in range(NQB):
                        # bias folded into the PSUM->SBUF cast (per-partition
                        # scalar add on DVE)
                        nc.vector.tensor_scalar_add(
                            out=dst[:, 0, ts(nb, 512)], in0=pb[nb],
                            scalar1=bqkvt[:, 0, brow:brow + 1])

                for t in range(NTC):
                    ps = ps1v.tile([128, HSW], F32, tag="psv", name="ps")
                    for kc in range(8):
                        nc.tensor.matmul(ps, xt[:, kc, ts(t, 128)], wv[:, kc, :],
                                         start=(kc == 0), stop=(kc == 7))
                    for h in range(HPC):
                        # v bias folded into the PSUM->SBUF cast on DVE
                        nc.vector.tensor_add(vaug[:, h, t, 0:HD],
                                             ps[:, ts(h, HD)], vb[:, ts(h, HD)])

            # ---- phase 2+3: attention with software-pipelined epilogues ----
            # Heads processed singly (qb outer). The normalization epilogue of
            # slot (qb, h) -- reciprocal via ACT exp(-ln d), PE broadcast, DVE
            # scale -- is DEFERRED until after the NEXT slot's score/ctx groups
            # are emitted: the static per-engine schedule then has the next
            # slot's matmuls between ctx-stop and the broadcast matmul, so the
            # PE never idles waiting on the reciprocal chain. Outproj for qb is
            # deferred two slots for the same reason.
            # Diagonal k-tiles are free-dim sliced to their valid query range
            # [128j, 512): only a 128x128 triangle add remains for masking.
            # PSUM: pss 2x2 + ctx 2 + misc 1 + ps1b 1 = 8 banks.
            with tc.tile_pool(name="pss", bufs=2, space="PSUM") as pss, \
                 tc.tile_pool(name="psc", bufs=2, space="PSUM") as psc, \
                 tc.tile_pool(name="psm", bufs=1, space="PSUM") as psm, \
                 tc.tile_pool(name="ps1b", bufs=1, space="PSUM") as ps1b, \
                 tc.tile_pool(name="attnp", bufs=3) as attnp, \
                 tc.tile_pool(name="outp", bufs=3) as outp:

                # Interleaved heads-2/3 q/k projection chunks: dense 9-matmul
                # accumulation runs that keep the PE HAM activity window busy
                # (warm clock) between attention slots. xt is resident, so no
                # re-DMA is needed.
                def qk_mc1_chunk(w, brow, dst, nb):
                    def run():
                        ps = ps1b.tile([128, 512], F32, tag="ps1b", name="ps1b")
                        for kc in range(8):
                            nc.tensor.matmul(ps, w[:, kc, 128:256],
                                             xt[:, kc, ts(nb, 512)],
                                             start=(kc == 0), stop=(kc == 7))
                        nc.vector.tensor_scalar_add(
                            out=dst[:, 1, ts(nb, 512)], in0=ps,
                            scalar1=bqkvt[:, 1, brow:brow + 1])
                    return run
                qk_units = [qk_mc1_chunk(w, brow, dst, nb)
                            for (w, brow, dst) in ((wq, 0, qT), (wk, 1, kT))
                            for nb in range(NQB)]



                def norm_epilogue(qb, h, cps):
                    def run():
                        mc, ro = h // 2, (h % 2) * HD
                        lnr = attnp.tile([1, 512], F32, tag="lnr", name="lnr")
                        nc.scalar.activation(out=lnr, in_=cps[HD:HD + 1, :],
                                             func=mybir.ActivationFunctionType.Ln)
                        rec = attnp.tile([1, 512], BF16, tag="rec", name="rec")
                        nc.scalar.activation(out=rec, in_=lnr, func=EXP,
                                             scale=-1.0)
                        bps = psm.tile([128, 512], F32, tag="m", name="bps")
                        nc.tensor.matmul(bps[0:HD, :], ones[0:1, 0:HD],
                                         rec[0:1, :], start=True, stop=True)
                        bsb = attnp.tile([HD, 512], BF16, tag="bsb", name="bsb")
                        nc.vector.tensor_copy(out=bsb, in_=bps[0:HD, :])
                        nc.vector.tensor_mul(
                            out=ctxT[ro:ro + HD, mc, ts(qb, 512)],
                            in0=cps[0:HD, :], in1=bsb)
                    return run

                def outproj(qb, half=None):
                    t0, t1 = 4 * qb, 4 * qb + 4
                    if half is not None:
                        t0, t1 = t0 + 2 * half, t0 + 2 * half + 2
                    def run():
                        for t in range(t0, t1):
                            osb = outp.tile([128, H], BF16, tag="osb", name="osb")
                            for n2 in range(2):
                                # Alternate across two PSUM banks (ps1b is idle
                                # once the qk bursts are done) so each outproj
                                # blob is a dense 8-matmul run - it both flows
                                # faster and re-warms the HAM clock.
                                ops = (psm if n2 == 0 else ps1b).tile(
                                    [128, 512], F32,
                                    tag="m" if n2 == 0 else "ps1b", name="ops")
                                nc.tensor.matmul(ops, ctxT[:, 0, ts(t, 128)],
                                                 wo[:, 0, ts(n2, 512)],
                                                 start=True, stop=False)
                                nc.tensor.matmul(ops, ctxT[:, 1, ts(t, 128)],
                                                 wo[:, 1, ts(n2, 512)],
                                                 start=False, stop=True)
                                nc.vector.tensor_copy(out=osb[:, ts(n2, 512)],
                                                      in_=ops)
                            nc.sync.dma_start(out=out_d[ts(t, 128), :], in_=osb)
                    return run

                # Slots run in DESCENDING qb order: the PE-dense big slots
                # execute early (inside the HAM warm-clock power budget), the
                # relatively ACT-heavy small slots land in the throttled
                # window, and the big outproj blocks flush mid-kernel instead
                # of at the tail.
                # Per-slot PE filler schedule: the 8 heads-2/3 qk chunks go on
                # the thin sec-1 slots (every slot is ACT-deficient at warm
                # clock). Unit q_nb/k_nb is needed before slot (nb,2).
                # ALL k units must land in sec 1: the first sec-2 slot (3,2)
                # reads every mc1 k-block. q unit nb is needed only by slot
                # (nb,2), so the later ones ride sec-2 slots as PE cover.
                uq = {nb: qk_units[nb] for nb in range(NQB)}
                uk = {nb: qk_units[NQB + nb] for nb in range(NQB)}
                fillers = {
                    (2, 0): uk[3], (2, 1): uk[2], (1, 0): uk[1],
                    (1, 1): uk[0], (0, 0): uq[3], (0, 1): uq[2],
                    (3, 2): uq[1], (2, 2): uq[0],
                }
                extra = {}
                deferred = []
                for qb, h in ([(q, hh) for q in reversed(range(NQB))
                               for hh in (0, 1)]
                              + [(q, hh) for q in reversed(range(NQB))
                                 for hh in (2, 3)]):
                    last_kb = 4 * qb + 3
                    if True:
                        mc, ro = h // 2, (h % 2) * HD
                        cps = psc.tile([128, 512], F32, tag="ctx", name="cps")
                        # Emit group PAIRS: 4 scores mms, 2 exps; the ctx mms
                        # of pair N are deferred until pair N+1's scores are
                        # emitted (one-pair lookahead), so the PE streams
                        # S(N) S(N+1) C(N) S(N+2) C(N+1)... and never stalls
                        # the full exp latency at a ctx group.
                        def emit_ctx(sets):
                            for g, sps, et in sets:
                                for u in range(2):
                                    kb = 2 * g + u
                                    j = kb - 4 * qb
                                    lo = 128 * j if j > 0 else 0
                                    nc.tensor.matmul(
                                        cps[0:HD + 1, lo:512],
                                        vaug[:, h, kb, :],
                                        et[:, u, lo:512],
                                        start=(kb == 0),
                                        stop=(kb == last_kb),
                                        skip_group_check=True)

                        pending = None
                        for gp in range(qb + 1):
                            diag = (gp == qb)
                            sets = []
                            for g in (2 * gp, 2 * gp + 1):
                                sps = pss.tile([128, 2, 512], F32, tag="s",
                                               name="sps")
                                et = attnp.tile([128, 2, 512], BF16, tag="et",
                                                name="et", bufs=4)
                                sets.append((g, sps, et))
                                for u in range(2):
                                    kb = 2 * g + u
                                    j = kb - 4 * qb
                                    lo = 128 * j if j > 0 else 0
                                    nc.tensor.matmul(
                                        sps[:, u, lo:512],
                                        kT[ro:ro + HD, mc, ts(kb, 128)],
                                        qT[ro:ro + HD, mc,
                                           qb * 512 + lo:(qb + 1) * 512],
                                        start=True, stop=True)
                                    if j >= 0:
                                        nc.vector.tensor_add(
                                            sps[:, u, 128 * j:128 * j + 128],
                                            sps[:, u, 128 * j:128 * j + 128],
                                            mbt)
                            for g, sps, et in sets:
                                if diag and g % 2 == 1:
                                    # j=2,3 tiles: one ACT over the union
                                    # range [256:512). The j=3 garbage cols
                                    # [256:384) hold stale scores (bounded),
                                    # exp of them is finite and never read.
                                    nc.scalar.activation(
                                        out=et[:, :, 256:512],
                                        in_=sps[:, :, 256:512],
                                        func=EXP, scale=0.125)
                                else:
                                    # off-diag, or diag j=0,1 (union is the
                                    # full range)
                                    nc.scalar.activation(out=et, in_=sps,
                                                         func=EXP, scale=0.125)
                            if pending is not None:
                                emit_ctx(pending)
                            pending = sets
                        emit_ctx(pending)
                        # flush one pending epilogue, then defer this slot's
                        while len(deferred) >= 2:
                            deferred.pop(0)()
                        # outproj halves (2 token chunks each) for qb+1 (whose
                        # norms have all flushed by now in descending order)
                        # queued BEFORE this slot's norm: the flush spreads
                        # their PE-dense work across the following slots.
                        if h == 3 and qb < NQB - 1:
                            deferred.append(outproj(qb + 1, 0))
                            deferred.append(outproj(qb + 1, 1))
                        deferred.append(norm_epilogue(qb, h, cps))
                        f = fillers.pop((qb, h), None)
                        if f is not None:
                            f()
                        f = extra.pop((qb, h), None)
                        if f is not None:
                            f()
                for fn in deferred:
                    fn()
                outproj(0, 0)()
                outproj(0, 1)()

    _split_multi_waits(nc)
    return nc


_NC_CACHE = []


def _get_nc():
    if not _NC_CACHE:
        _NC_CACHE.append(_build())
    return _NC_CACHE[0]


def _triangle_mask() -> np.ndarray:
    """mbt[p, f] = 0 where p <= f (key p attends to query f), else NEG.
    Applied to the 128x128 leading-diagonal corner of each diagonal k-tile."""
    p = np.arange(128)[:, None]
    f = np.arange(128)[None, :]
    return np.where(p <= f, 0.0, NEG).astype(np.float32)


def _in_maps(inputs: dict) -> list[dict]:
    bf16 = ml_dtypes.bfloat16
    x = np.asarray(inputs["hidden_states"], dtype=np.float32).astype(bf16)
    Wq = np.asarray(inputs["Wq"], dtype=np.float32).astype(bf16)
    Wk = np.asarray(inputs["Wk"], dtype=np.float32).astype(bf16)
    Wv = np.asarray(inputs["Wv"], dtype=np.float32).astype(bf16)
    Wo = np.asarray(inputs["Wo"], dtype=np.float32).astype(bf16)
    bq = np.asarray(inputs["bq"], dtype=np.float32).astype(bf16)
    bk = np.asarray(inputs["bk"], dtype=np.float32).astype(bf16)
    bv = np.asarray(inputs["bv"], dtype=np.float32).astype(bf16)

    xts = [np.ascontiguousarray(x[b].T) for b in range(B)]
    mbt = _triangle_mask()

    def wlayout(wt, c):
        # [c*128, n] -> [128, c, n] so per-partition DMA runs are contiguous
        return np.ascontiguousarray(
            wt.reshape(c, 128, wt.shape[1]).transpose(1, 0, 2))

    bqf = np.asarray(inputs["bq"], dtype=np.float32)
    bkf = np.asarray(inputs["bk"], dtype=np.float32)
    bvf = np.asarray(inputs["bv"], dtype=np.float32)
    maps = []
    for c in range(NCORES):
        b, hg = c // 4, c % 4
        hs = slice(hg * HSW, (hg + 1) * HSW)
        # [p, mc, q|k] fp32 per-row bias for the DVE tensor_scalar add
        bqkvt = np.ascontiguousarray(
            np.stack([bqf[hs].reshape(2, 128), bkf[hs].reshape(2, 128)],
                     axis=-1).transpose(1, 0, 2))
        maps.append({
            "xt": xts[b],
            "wq": wlayout(np.ascontiguousarray(Wq[hs, :].T), 8),
            "wk": wlayout(np.ascontiguousarray(Wk[hs, :].T), 8),
            "wv": wlayout(np.ascontiguousarray(Wv[hs, :].T), 8),
            "wo": wlayout(np.ascontiguousarray(Wo[:, hs].T), 2),
            "vb": np.ascontiguousarray(
                np.broadcast_to(bvf[hs][None, :], (128, HSW))),
            "bqkvt": bqkvt,
            "mb": mbt,
        })
    return maps


def run(inputs: dict, **spmd_kwargs):
    """Returns (full_output, BassKernelResults)."""
    nc = _get_nc()
    res = run_bass_kernel_spmd(nc, _in_maps(inputs), list(range(NCORES)),
                               **spmd_kwargs)
    bo = np.asarray(inputs["bo"], dtype=np.float32)
    out = np.empty((B, S, H), dtype=np.float32)
    for b in range(B):
        acc = res.results[4 * b]["out"].astype(np.float32)
        for hg in range(1, 4):
            acc = acc + res.results[4 * b + hg]["out"].astype(np.float32)
        out[b] = acc + bo
    return out, res


def kernel(**inputs) -> np.ndarray:
    out, _ = run(inputs)
    return out
